# revision 17
# baseline (speedup 1.0000x reference)
"""Trainium2 Bass kernel for 16-head MultiHeadAttention (B=2, T=2048, D=1024).

Sharding (8 NeuronCores): core c handles batch b = c//4 and head group
g = c%4 (heads 4g..4g+3).  Each core computes Q/K/V projections for its 4
heads, attention, and a partial output projection against its 256 rows of
W_O.  The host sums the 4 partials per batch and adds b_O (row-parallel TP;
the all-reduce is folded into the unshard step).

Device layout notes:
 - The host pre-transposes x to x^T [D, T] so the contraction dim (features)
   lands on SBUF partitions without any on-device transposes (fp32 has no
   DMA-transpose path).
 - Attention is computed in the S^T = K @ Q^T orientation: the softmax
   denominator is then a partition-axis sum, which the PE produces for free
   via a ones-column appended to V (out = [V|1]^T @ P^T gives O^T rows 0..63
   and the denominator in row 64).
 - Per head pair (2 heads of 64), weights are stacked to fill 128 partitions.
 - Matmuls run as float32r (full fp32 storage, 1 cycle/row at N>=512).
"""

import os
import sys

import numpy as np

for _p in ("/opt/trn_rl_repo", "/root/.axon_site/_ro/trn_rl_repo"):
    if os.path.isdir(_p) and _p not in sys.path:
        sys.path.insert(0, _p)

import concourse.bass as bass
import concourse.mybir as mybir
import concourse.tile as tile
from concourse import bacc
from concourse.bass_utils import run_bass_kernel_spmd
from concourse.masks import make_identity

F32 = mybir.dt.float32
F32R = mybir.dt.float32r
BF16 = mybir.dt.bfloat16
AF = mybir.ActivationFunctionType

B, TQ, TK = 2, 2048, 2048
D = 1024          # model dim == x_to/x_from feature dim
H, DH = 16, 64
N_CORES = 8
HEADS_PER_CORE = 4   # one batch per core
HP = 2               # head pairs per core (2 heads of 64 stacked -> 128)

TT = 512             # t-tile (moving free dim)
N_TT = TQ // TT      # 4
N_SC = TK // 128     # 16 s-chunks
N_FC = D // 128      # 8 f-chunks

USE_F32R = False
USE_BF16 = True
DT = BF16 if USE_BF16 else F32

_CACHED = {}


def _r(ap):
    return ap.bitcast(F32R) if USE_F32R else ap


def build_program():
    nc = bacc.Bacc(
        "TRN2", target_bir_lowering=False, debug=False, num_devices=N_CORES
    )

    xt_to = nc.dram_tensor("xt_to", [D, TQ], DT, kind="ExternalInput")
    xt_from = nc.dram_tensor("xt_from", [D, TK], DT, kind="ExternalInput")
    wq = nc.dram_tensor("wq", [D, 256], DT, kind="ExternalInput")
    wk = nc.dram_tensor("wk", [D, 256], DT, kind="ExternalInput")
    wv = nc.dram_tensor("wv", [D, 256], DT, kind="ExternalInput")
    bq = nc.dram_tensor("bq", [128, 2], F32, kind="ExternalInput")
    bk = nc.dram_tensor("bk", [128, 2], F32, kind="ExternalInput")
    bv = nc.dram_tensor("bv", [128, 2], F32, kind="ExternalInput")
    wot = nc.dram_tensor("wot", [128, 2, 1024], DT, kind="ExternalInput")
    # head-selector row for broadcasting denominators: cols 0:128 select
    # head 0 (1.0 at 0:64), cols 128:256 select head 1 (1.0 at 192:256)
    esel = nc.dram_tensor("esel", [1, 256], F32, kind="ExternalInput")
    out = nc.dram_tensor("out", [TQ, D], F32, kind="ExternalOutput")

    with tile.TileContext(nc) as tc:
        with (
            tc.tile_pool(name="wpool", bufs=1) as wpool,
            tc.tile_pool(name="xpool", bufs=3) as xpool,
            tc.tile_pool(name="actpool", bufs=1) as actpool,
            tc.tile_pool(name="ptpool", bufs=3) as ptpool,
            tc.tile_pool(name="misc", bufs=2) as misc,
            tc.tile_pool(name="psmm", bufs=3, space="PSUM") as psmm,
            tc.tile_pool(name="psacc", bufs=2, space="PSUM") as psacc,
            tc.tile_pool(name="pstr", bufs=2, space="PSUM") as pstr,
        ):
            # ---- constants & weights -------------------------------------
            ident = wpool.tile([128, 128], DT)
            make_identity(nc, ident[:])

            esel_sb = wpool.tile([1, 256], F32)
            nc.sync.dma_start(esel_sb[:], esel[:])

            wq_sb = wpool.tile([128, N_FC, 256], DT)
            wk_sb = wpool.tile([128, N_FC, 256], DT)
            wv_sb = wpool.tile([128, N_FC, 256], DT)
            nc.sync.dma_start(wq_sb[:], wq.rearrange("(c p) d -> p c d", p=128))
            nc.sync.dma_start(wk_sb[:], wk.rearrange("(c p) d -> p c d", p=128))
            nc.sync.dma_start(wv_sb[:], wv.rearrange("(c p) d -> p c d", p=128))

            bq_sb = wpool.tile([128, 2], F32)
            bk_sb = wpool.tile([128, 2], F32)
            bv_sb = wpool.tile([128, 2], F32)
            nc.sync.dma_start(bq_sb[:], bq[:])
            nc.sync.dma_start(bk_sb[:], bk[:])
            nc.sync.dma_start(bv_sb[:], bv[:])

            wot_sb = wpool.tile([128, 2, 1024], DT)
            nc.sync.dma_start(wot_sb[:], wot[:])

            # ---- persistent activations ----------------------------------
            # Q^T / K^T per head pair: [d(2 heads stacked)=128, T]
            qt_sb = [
                actpool.tile([128, TQ], DT, name=f"qt{hp}") for hp in range(HP)
            ]
            kt_sb = [
                actpool.tile([128, TK], DT, name=f"kt{hp}") for hp in range(HP)
            ]
            # V natural + ones columns: [s=128, sc, (V_h0|1|V_h1|1)=130]
            vn_sb = [
                actpool.tile([128, N_SC, 130], DT, name=f"vn{hp}")
                for hp in range(HP)
            ]
            # O^T: unnormalized after attention, normalized in place after
            ot_sb = [
                actpool.tile([128, TQ], DT, name=f"ot{hp}") for hp in range(HP)
            ]
            # softmax denominators, packed along the free dim of partition 0:
            # (hp, h) lives at free offset (2*hp+h)*TQ; reciprocal in place
            rec_all = actpool.tile([1, 4 * TQ], F32, name="rec_all")

            xt_to_r = xt_to.rearrange("(c p) t -> p c t", p=128)
            xt_from_r = xt_from.rearrange("(c p) t -> p c t", p=128)

            # ---- QKV projections (+ V transpose fused) -------------------
            for tt in range(N_TT):
                ts = bass.ts(tt, TT)
                xto_t = xpool.tile([128, N_FC, TT], DT, tag="xt", name="xto_t")
                nc.sync.dma_start(xto_t[:], xt_to_r[:, :, ts])
                xfr_t = xpool.tile([128, N_FC, TT], DT, tag="xt", name="xfr_t")
                nc.sync.dma_start(xfr_t[:], xt_from_r[:, :, ts])

                for hp in range(HP):
                    dsl = bass.ts(hp, 128)
                    ps_q = psmm.tile([128, TT], F32, tag="mm", name="ps_q")
                    for fc in range(N_FC):
                        nc.tensor.matmul(
                            ps_q[:],
                            _r(wq_sb[:, fc, dsl]),
                            _r(xto_t[:, fc, :]),
                            start=(fc == 0),
                            stop=(fc == N_FC - 1),
                        )
                    nc.vector.tensor_scalar_add(
                        qt_sb[hp][:, ts], ps_q[:], bq_sb[:, hp : hp + 1]
                    )

                    ps_k = psmm.tile([128, TT], F32, tag="mm", name="ps_k")
                    for fc in range(N_FC):
                        nc.tensor.matmul(
                            ps_k[:],
                            _r(wk_sb[:, fc, dsl]),
                            _r(xfr_t[:, fc, :]),
                            start=(fc == 0),
                            stop=(fc == N_FC - 1),
                        )
                    nc.vector.tensor_scalar_add(
                        kt_sb[hp][:, ts], ps_k[:], bk_sb[:, hp : hp + 1]
                    )

                    ps_v = psmm.tile([128, TT], F32, tag="mm", name="ps_v")
                    for fc in range(N_FC):
                        nc.tensor.matmul(
                            ps_v[:],
                            _r(wv_sb[:, fc, dsl]),
                            _r(xfr_t[:, fc, :]),
                            start=(fc == 0),
                            stop=(fc == N_FC - 1),
                        )
                    vtt = misc.tile([128, TT], DT, tag="vtt", name="vtt")
                    nc.vector.tensor_scalar_add(
                        vtt[:], ps_v[:], bv_sb[:, hp : hp + 1]
                    )
                    # V^T [d,s] chunk -> V natural [s,d] via PE transpose
                    for j in range(TT // 128):
                        sc = tt * (TT // 128) + j
                        ps_t = pstr.tile([128, 128], DT, tag="tr", name="ps_t")
                        nc.tensor.transpose(
                            ps_t[:], vtt[:, bass.ts(j, 128)], ident[:]
                        )
                        nc.vector.tensor_copy(
                            vn_sb[hp][:, sc, 0:64], ps_t[:, 0:64]
                        )
                        nc.vector.tensor_copy(
                            vn_sb[hp][:, sc, 65:129], ps_t[:, 64:128]
                        )

            for hp in range(HP):
                nc.vector.memset(vn_sb[hp][:, :, 64], 1.0)
                nc.vector.memset(vn_sb[hp][:, :, 129], 1.0)

            # ---- attention (S^T orientation) -----------------------------
            for hp in range(HP):
                for h in range(2):
                    hb = 64 * h
                    vb = 65 * h
                    for tt in range(N_TT):
                        ts = bass.ts(tt, TT)
                        ps_o = psacc.tile([65, TT], F32, tag="acc", name="ps_o")
                        for sc in range(N_SC):
                            ps_s = psmm.tile([128, TT], F32, tag="mm", name="ps_s")
                            nc.tensor.matmul(
                                ps_s[:],
                                _r(kt_sb[hp][hb : hb + 64, bass.ts(sc, 128)]),
                                _r(qt_sb[hp][hb : hb + 64, ts]),
                                start=True,
                                stop=True,
                            )
                            pt = ptpool.tile([128, TT], DT, tag="pt", name="pt")
                            nc.scalar.activation(pt[:], ps_s[:], AF.Exp)
                            nc.tensor.matmul(
                                ps_o[:],
                                _r(vn_sb[hp][:, sc, vb : vb + 65]),
                                _r(pt[:]),
                                start=(sc == 0),
                                stop=(sc == N_SC - 1),
                            )
                        nc.vector.tensor_copy(
                            ot_sb[hp][hb : hb + 64, ts], ps_o[0:64, :]
                        )
                        off = (2 * hp + h) * TQ + tt * TT
                        nc.vector.tensor_copy(
                            rec_all[0:1, off : off + TT], ps_o[64:65, :]
                        )

            # ---- normalize: O^T *= (1/den) broadcast across partitions ---
            for hp in range(HP):
                hoff = 2 * hp * TQ
                nc.vector.reciprocal(
                    rec_all[0:1, hoff : hoff + 2 * TQ],
                    rec_all[0:1, hoff : hoff + 2 * TQ],
                )
                for tt in range(N_TT):
                    ts = bass.ts(tt, TT)
                    ps_r = psmm.tile([128, TT], F32, tag="mm", name="ps_r")
                    for h in range(2):
                        off = (2 * hp + h) * TQ + tt * TT
                        nc.tensor.matmul(
                            ps_r[:],
                            _r(esel_sb[0:1, bass.ts(h, 128)]),
                            _r(rec_all[0:1, off : off + TT]),
                            start=(h == 0),
                            stop=(h == 1),
                        )
                    r_sb = misc.tile([128, TT], DT, tag="rsb", name="r_sb")
                    nc.vector.tensor_copy(r_sb[:], ps_r[:])
                    nc.vector.tensor_mul(
                        ot_sb[hp][:, ts], ot_sb[hp][:, ts], r_sb[:]
                    )

            # ---- output projection (partial over this core's 256 rows) ---
            for tc_ in range(TQ // 128):
                tsl = bass.ts(tc_, 128)
                for ec in range(2):
                    esl = bass.ts(ec, TT)
                    ps_out = psmm.tile([128, TT], F32, tag="mm", name="ps_out")
                    for hp in range(HP):
                        nc.tensor.matmul(
                            ps_out[:],
                            _r(ot_sb[hp][:, tsl]),
                            _r(wot_sb[:, hp, esl]),
                            start=(hp == 0),
                            stop=(hp == HP - 1),
                        )
                    o_t = misc.tile([128, TT], F32, tag="out", name="o_t")
                    nc.scalar.activation(o_t[:], ps_out[:], AF.Copy)
                    nc.sync.dma_start(out[tsl, esl], o_t[:])

    nc.compile()
    return nc


def _prep_in_maps(x_to, x_from, Wq, bq, Wk, bk, Wv, bv, Wo):
    scale = 1.0 / np.sqrt(np.float32(DH))
    # [H, D, DH] -> [D, H*DH] with column h*DH+d
    wq_f = np.ascontiguousarray(Wq.transpose(1, 0, 2).reshape(D, H * DH)) * scale
    wk_f = np.ascontiguousarray(Wk.transpose(1, 0, 2).reshape(D, H * DH))
    wv_f = np.ascontiguousarray(Wv.transpose(1, 0, 2).reshape(D, H * DH))
    bq_f = bq.reshape(H * DH) * scale
    bk_f = bk.reshape(H * DH)
    bv_f = bv.reshape(H * DH)

    xt_to = np.ascontiguousarray(x_to.transpose(0, 2, 1))    # [B, D, TQ]
    xt_from = np.ascontiguousarray(x_from.transpose(0, 2, 1))

    def f32(a):
        return np.ascontiguousarray(a, dtype=np.float32)

    if USE_BF16:
        import ml_dtypes

        def fdt(a):
            return np.ascontiguousarray(a, dtype=ml_dtypes.bfloat16)
    else:
        fdt = f32

    esel = np.zeros((1, 256), np.float32)
    esel[0, 0:64] = 1.0
    esel[0, 192:256] = 1.0

    in_maps = []
    for c in range(N_CORES):
        b, g = divmod(c, HEADS_PER_CORE)
        cs = slice(g * 256, (g + 1) * 256)
        in_maps.append(
            {
                "xt_to": fdt(xt_to[b]),
                "xt_from": fdt(xt_from[b]),
                "wq": fdt(wq_f[:, cs]),
                "wk": fdt(wk_f[:, cs]),
                "wv": fdt(wv_f[:, cs]),
                # [256] -> [2 pairs, 128] -> [128, 2]
                "bq": f32(bq_f[cs].reshape(2, 128).T),
                "bk": f32(bk_f[cs].reshape(2, 128).T),
                "bv": f32(bv_f[cs].reshape(2, 128).T),
                # Wo[:, cs].T = [256, 1024] -> [2, 128, 1024] -> [128, 2, 1024]
                "wot": fdt(
                    np.ascontiguousarray(Wo[:, cs].T)
                    .reshape(2, 128, 1024)
                    .transpose(1, 0, 2)
                ),
                "esel": esel,
            }
        )
    return in_maps


LAST_EXEC_TIME_NS = None
LAST_TRACE = None


def kernel(x_to, x_from, Wq, bq, Wk, bk, Wv, bv, Wo, bo):
    global LAST_EXEC_TIME_NS, LAST_TRACE
    if "nc" not in _CACHED:
        _CACHED["nc"] = build_program()
    nc = _CACHED["nc"]

    in_maps = _prep_in_maps(
        np.asarray(x_to), np.asarray(x_from), np.asarray(Wq), np.asarray(bq),
        np.asarray(Wk), np.asarray(bk), np.asarray(Wv), np.asarray(bv),
        np.asarray(Wo),
    )
    res = run_bass_kernel_spmd(nc, in_maps, list(range(N_CORES)))
    LAST_EXEC_TIME_NS = res.exec_time_ns
    LAST_TRACE = res.instructions_and_trace

    out = np.zeros((B, TQ, D), dtype=np.float32)
    for c in range(N_CORES):
        out[c // HEADS_PER_CORE] += res.results[c]["out"]
    out += np.asarray(bo, dtype=np.float32)
    return out
